# revision 5
# baseline (speedup 1.0000x reference)
"""DeepTensorNeuralNetwork (DTNN / gnn_message_passing) Trainium2 kernel.

Math (per reference):
    d_sum = distance.sum(axis=2)                                  # (B,N,R)
    for l in 0..2:
        cf = x @ Wcf[l].T + bcf[l]                                # (B,N,H)
        df = d_sum @ Wdf[l].T + N*bdf[l]                          # (B,N,H)
        h  = (cf*df) @ Wfc[l].T                                   # (B,N,F)
        x  = h + tanh(h)
    g = x.sum(axis=1); out = (g @ fc0.T + b0) @ ow.T + ob         # (B,1)

Strategy: data-parallel over batch across 8 NeuronCores (8 batches/core),
distance cast to fp16 on host (j-sum quantization ~1e-4 rel, gate 2e-2).

v2 layout of work:
- dist stream: lo cols on the sync HWDGE ring, hi cols on the gpsimd ring
  (the scalar queue's ring starts ~2.3us late behind ACT_TABLE_LOAD);
  wpack rides the tensor ring so the two dist rings carry dist only.
  b0/b1 stream in two half-chunks each so DVE folding starts ~2.5us
  earlier; b7 also (its last half gates the tail).
- j-folds on DVE in fp16 2x mode stop at 256 cols = (j4, r); the last two
  2:1 levels are absorbed into the df matmul: two PE transposes per batch
  -> dsT (t-major [128, 2*G*N]) and df = mm(wdf2, a) + mm(wdf2, b).
- per layer, the c0 half's m=cf*df runs on GpSimd, c1 on DVE.
- heads: the l2 copy/tanh ACT ops run per batch with accum_out, giving
  g_c/g_t [128, 8] column sums for free; one final fp32 matmul
  out = head32 @ (g_c + g_t) replaces all head matmuls + DVE reduces.
- groups (4,1), (4,1)... GROUPS=( (0,1,2,3), (4,5,6), (7,) ): the last
  group is a single-batch latency path (PSUM-fused stt ops).
"""

import numpy as np

B, N, F, R, H = 64, 128, 128, 64, 256
L = 3
NCORES = 8
BL = B // NCORES   # batches per core
GROUPS = ((0, 1, 2, 3), (4, 5, 6), (7,))

# wpack layout, fp32 columns (bf-cols = fp16-element columns of the
# bitcast view at 2x the fp32 column index):
#   [0, 384)      wcf lhsT f16 : bf-col l*H+h           = Wcf_w[l, h, f]
#   [384, 768)    wfc lhsT f16 : bf-col (l*2+c)*F+f     = Wfc_w[l, f, c*128+hc]
#   [768, 774)    cf bias fp32 : col l*2+c              = Wcf_b[l, c*128+h]
#   [774, 780)    df bias fp32 : col l*2+c              = N * Wdf_b[l, c*128+h]
#   [780, 781)    head  fp32   : col 0                  = (out_w @ fc0_w)[0, f]
#   [784, 1168)   wdf2 lhsT f16: bf-col l*H+h, row j2*64+r = Wdf_w[l, h, r]
#   [1168, 1680)  x f16        : bf-col b*N+n           = x[b_local, n, f]
#   [1680, 1744)  identity f16-packed
BCF_OFF = 768
BDF_OFF = 774
HEAD_OFF = 780
WDF_OFF = 784
XOFF = 1168
IDOFF = 1680
WCOLS = 1744

_CACHE = {}


def _build_program():
    import concourse.bass as bass
    from concourse import bacc
    import concourse.tile as tile
    from concourse import mybir
    from concourse import hw_specs

    # The Tile scheduler orders the (runtime in-order) engine queues from a
    # simulation that models HBM at ~332 GB/s; this device streams ~420.
    # Build with the measured rate so simulated data-arrival (and hence the
    # queue order) matches hardware; restored right after the build.
    _dma_cycle_prev = hw_specs.TRN2Spec.DMA_CYCLE
    hw_specs.TRN2Spec.DMA_CYCLE = 1e9 / (420e9 / 128)
    try:
        return _build_program_inner(bass, bacc, tile, mybir)
    finally:
        hw_specs.TRN2Spec.DMA_CYCLE = _dma_cycle_prev


def _build_program_inner(bass, bacc, tile, mybir):

    f32 = mybir.dt.float32
    f16 = mybir.dt.float16
    AF = mybir.ActivationFunctionType

    nc = bacc.Bacc("TRN2")
    dist = nc.declare_dram_parameter("dist", [BL, N, N * R], f16, isOutput=False)
    wpack = nc.declare_dram_parameter("wpack", [128, WCOLS], f32, isOutput=False)
    out_ext = nc.declare_dram_parameter("out", [BL, 1], f32, isOutput=True)

    with tile.TileContext(nc) as tc:
        with (
            tc.tile_pool(name="consts", bufs=1) as consts,
            tc.tile_pool(name="dist", bufs=4) as dist_pool,
            tc.tile_pool(name="fold", bufs=2) as fold_pool,
            tc.tile_pool(name="dsum", bufs=2) as dsum_pool,
            tc.tile_pool(name="work", bufs=2) as work,
            tc.tile_pool(name="psA", bufs=1, space="PSUM") as psA,
            tc.tile_pool(name="psB", bufs=1, space="PSUM") as psB,
            tc.tile_pool(name="psS", bufs=1, space="PSUM") as psS,
        ):
            # ---- DMA: dist on sync+gpsimd rings, wpack on tensor ---------
            dist_tiles = {}

            def start_dist_dma(b, hf=None):
                if b in dist_tiles:
                    t = dist_tiles[b]
                else:
                    tag = "dist67" if b >= 6 else "dist"
                    bufs = 2 if b >= 6 else 4
                    t = dist_pool.tile([N, N * R], f16, tag=tag,
                                       name="dist_t", bufs=bufs)
                    dist_tiles[b] = t
                dflat = dist[b, :, :]
                lo = 0 if hf in (None, 0) else 4096
                hi = 8192 if hf in (None, 1) else 4096
                # sync ring starts ~2.3us before the scalar ring (the scalar
                # queue sits behind ACT_TABLE_LOAD preamble) -> give sync
                # ~51.6% of each chunk so both rings deliver chunks together.
                mid = lo + ((hi - lo) * 33 // 64) // 32 * 32
                nc.sync.dma_start(out=t[:, lo:mid], in_=dflat[:, lo:mid])
                nc.scalar.dma_start(out=t[:, mid:hi], in_=dflat[:, mid:hi])

            start_dist_dma(0, 0)
            start_dist_dma(0, 1)
            start_dist_dma(1, 0)
            start_dist_dma(1, 1)

            wp = consts.tile([128, WCOLS], f32)
            hwc = WCOLS // 2
            nc.gpsimd.dma_start(out=wp[:, 0:hwc], in_=wpack[:, 0:hwc])
            nc.gpsimd.dma_start(out=wp[:, hwc:WCOLS], in_=wpack[:, hwc:WCOLS])
            wb = wp.bitcast(f16)  # (128, 2*WCOLS) f16 view
            ident = wb[:, 2 * IDOFF : 2 * IDOFF + 128]
            out_acc = consts.tile([1, BL], f32)
            g_c = consts.tile([128, BL], f32, name="g_c")
            g_t = consts.tile([128, BL], f32, name="g_t")

            start_dist_dma(2)
            start_dist_dma(3)

            def wcf_l(l, c):
                o = l * H + c * 128
                return wb[:, o : o + 128]

            def wdf_l(l, c):
                o = 2 * WDF_OFF + l * H + c * 128
                return wb[:, o : o + 128]

            def wfc_l(l, c):
                o = 2 * 384 + (l * 2 + c) * F
                return wb[:, o : o + F]

            def bcf_l(l, c):
                o = BCF_OFF + l * 2 + c
                return wp[:, o : o + 1]

            def bdf_l(l, c):
                o = BDF_OFF + l * 2 + c
                return wp[:, o : o + 1]

            head32 = wp[:, HEAD_OFF : HEAD_OFF + 1]

            def xcols(b0, b1):
                return wb[:, 2 * XOFF + b0 * N : 2 * XOFF + b1 * N]

            # ---- folds (8192 -> 256 = (j4, r), fp16 2x) -----------------
            dsums = {}

            def get_dsum(b):
                if b not in dsums:
                    dsums[b] = dsum_pool.tile([N, 256], f16, tag="dsum",
                                              name="dsum", bufs=8)
                return dsums[b]

            def fold_full(b, then_dma=()):
                """One tree 8192 -> 256 cols (j4, r): 5 DVE ops."""
                src = dist_tiles.pop(b)
                for args in then_dma:
                    start_dist_dma(*args)
                dsum = get_dsum(b)
                s = fold_pool.tile([N, 4096], f16, tag="s", name="s")
                nc.vector.tensor_add(s, src[:, 0:4096], src[:, 4096:8192])
                t = fold_pool.tile([N, 2048], f16, tag="t", name="t")
                cur, other, w = s, t, 2048
                while w >= 256:
                    dst = dsum if w == 256 else other[:, 0:w]
                    nc.vector.tensor_add(dst, cur[:, 0:w], cur[:, w : 2 * w])
                    cur, other = other, cur
                    w //= 2
                dsums[b] = dsum

            def fold_half(b, hf, pop=False, then_dma=()):
                """Half-tree (j-range hf) -> dsum[:, hf*128 : hf*128+128]."""
                src_t = dist_tiles[b]
                if pop:
                    dist_tiles.pop(b)
                for args in then_dma:
                    start_dist_dma(*args)
                dsum = get_dsum(b)
                off = hf * 4096
                s = fold_pool.tile([N, 2048], f16, tag=f"hs{hf}", name="hs")
                nc.vector.tensor_add(s, src_t[:, off : off + 2048],
                                     src_t[:, off + 2048 : off + 4096])
                t = fold_pool.tile([N, 1024], f16, tag=f"ht{hf}", name="ht")
                cur, other, w = s, t, 1024
                while w >= 128:
                    dst = dsum[:, hf * 128 : hf * 128 + 128] if w == 128 \
                        else other[:, 0:w]
                    nc.vector.tensor_add(dst, cur[:, 0:w], cur[:, w : 2 * w])
                    cur, other = other, cur
                    w //= 2

            # ---- group state / layer pipeline ---------------------------
            gstate = {}

            def ps_pool(gi, c=0):
                if gi == 0:
                    return psA
                if gi == 1:
                    return psB
                return psA if c == 0 else psB

            def emit_trs(gi):
                """dsum[b] (i, (j4,r)) -> dsT t-major [128, 2*G*N] f16."""
                bs = GROUPS[gi]
                G = len(bs)
                NG = G * N
                dsT = dsum_pool.tile([128, 2 * 4 * N], f16, tag="dsT",
                                     name=f"dsT{gi}")
                dv = dsT.rearrange("p (t gn) -> p t gn", t=2)
                for k, b in enumerate(bs):
                    ds = dsums.pop(b)
                    trp = psS.tile([128, 256], f16, tag="tr", name="trp")
                    nc.tensor.transpose(trp[:, 0:128], ds[:, 0:128], ident)
                    nc.tensor.transpose(trp[:, 128:256], ds[:, 128:256], ident)
                    nc.scalar.activation(
                        out=dv[:, :, k * N : (k + 1) * N],
                        in_=trp.rearrange("p (t n) -> p t n", t=2),
                        func=AF.Copy,
                    )
                gstate[gi] = {"dsTa": dsT[:, 0 : 4 * N][:, 0:NG],
                              "dsTb": dsT[:, 4 * N : 8 * N][:, 0:NG],
                              "NG": NG, "bs": bs,
                              "xc": xcols(bs[0], bs[-1] + 1)}

            def emit_df(gi, l, c, st):
                NG = st["NG"]
                dfp = ps_pool(gi, 1).tile([128, 4 * N], f32, tag="df",
                                          name="dfp")[:, 0:NG]
                nc.tensor.matmul(dfp, wdf_l(l, c), st["dsTa"],
                                 start=True, stop=False)
                nc.tensor.matmul(dfp, wdf_l(l, c), st["dsTb"],
                                 start=False, stop=True)
                return dfp

            def emit_layer(gi, l):
                """Throughput path: ACT bias copies, muls on GP (c0) / DVE
                (c1)."""
                st = gstate[gi]
                NG = st["NG"]
                ms = []
                for c in range(2):
                    cfp = ps_pool(gi, 0).tile([128, 4 * N], f32, tag="cf",
                                              name="cfp")[:, 0:NG]
                    if l == 0:
                        nc.tensor.matmul(cfp, wcf_l(l, c), st["xc"],
                                         start=True, stop=True)
                    else:
                        nc.tensor.matmul(cfp, wcf_l(l, c), st["hsb"],
                                         start=True, stop=False)
                        nc.tensor.matmul(cfp, wcf_l(l, c), st["th"],
                                         start=False, stop=True)
                    cfs = work.tile([128, 4 * N], f16, tag=f"cfs{gi % 2}{c}",
                                    name="cfs")[:, 0:NG]
                    nc.scalar.activation(out=cfs, in_=cfp, func=AF.Identity,
                                         bias=bcf_l(l, c))
                    dfp = emit_df(gi, l, c, st)
                    dfs = work.tile([128, 4 * N], f16, tag=f"dfs{gi % 2}{c}",
                                    name="dfs")[:, 0:NG]
                    nc.scalar.activation(out=dfs, in_=dfp, func=AF.Identity,
                                         bias=bdf_l(l, c))
                    m = work.tile([128, 4 * N], f16, tag=f"m{gi % 2}{c}",
                                  name="m")[:, 0:NG]
                    if c == 0:
                        nc.gpsimd.tensor_mul(m, cfs, dfs)
                    else:
                        nc.vector.tensor_mul(m, cfs, dfs)
                    ms.append(m)
                _emit_h(gi, l, ms)

            def _emit_h(gi, l, ms):
                st = gstate[gi]
                NG = st["NG"]
                hpool = psA if gi == 0 else psB
                hp = hpool.tile([F, 4 * N], f32, tag="h", name="hp")[:, 0:NG]
                nc.tensor.matmul(hp, wfc_l(l, 0), ms[0], start=True, stop=False)
                nc.tensor.matmul(hp, wfc_l(l, 1), ms[1], start=False, stop=True)
                if l < L - 1:
                    hsb = work.tile([F, 4 * N], f16, tag=f"hsb{gi % 2}",
                                    name="hsb")[:, 0:NG]
                    nc.scalar.activation(out=hsb, in_=hp, func=AF.Copy)
                    th = work.tile([F, 4 * N], f16, tag=f"th{gi % 2}",
                                   name="th")[:, 0:NG]
                    nc.scalar.activation(out=th, in_=hp, func=AF.Tanh)
                    st["hsb"], st["th"] = hsb, th
                else:
                    _emit_l2_accum(st["bs"], hp)

            def _emit_l2_accum(bs, hp):
                """Per-batch ACT copy/tanh with accum_out -> g_c/g_t cols."""
                for k, b in enumerate(bs):
                    sl = hp[:, k * N : (k + 1) * N]
                    scr = work.tile([F, N], f16, tag="l2scr", name="l2scr",
                                    bufs=1)
                    nc.scalar.activation(out=scr, in_=sl, func=AF.Copy,
                                         accum_out=g_c[:, b : b + 1])
                    scr2 = work.tile([F, N], f16, tag="l2scr2", name="l2scr2",
                                     bufs=1)
                    nc.scalar.activation(out=scr2, in_=sl, func=AF.Tanh,
                                         accum_out=g_t[:, b : b + 1])

            # ---- tail-group latency path (single batch b7) --------------
            gl = {}

            def emit_glast_pre(gi):
                bs = GROUPS[gi]
                NG = len(bs) * N
                st = gl.setdefault(gi, {})
                st["NG"], st["bs"] = NG, bs
                cfs0 = []
                for c in range(2):
                    cfp = ps_pool(gi, c).tile([128, 4 * N], f32, tag="cf",
                                              name="cfpL")[:, 0:NG]
                    nc.tensor.matmul(cfp, wcf_l(0, c), xcols(bs[0], bs[-1] + 1),
                                     start=True, stop=True)
                    cs = work.tile([128, N], f16, tag=f"glcf{c}",
                                   name="glcfs", bufs=1)[:, 0:NG]
                    nc.scalar.activation(out=cs, in_=cfp, func=AF.Identity,
                                         bias=bcf_l(0, c))
                    cfs0.append(cs)
                st["cfs0"] = cfs0

            def emit_glast_trs(gi):
                st = gl[gi]
                bs, NG = st["bs"], st["NG"]
                dsT = dsum_pool.tile([128, 256], f16, tag="dsTL",
                                     name="dsTL")
                b = bs[0]
                ds = dsums.pop(b)
                trp = psS.tile([128, 256], f16, tag="tr", name="trpL")
                nc.tensor.transpose(trp[:, 0:128], ds[:, 0:128], ident)
                nc.tensor.transpose(trp[:, 128:256], ds[:, 128:256], ident)
                nc.scalar.activation(out=dsT, in_=trp, func=AF.Copy)
                st["dsTa"] = dsT[:, 0:128]
                st["dsTb"] = dsT[:, 128:256]

            def emit_glast_dfs(gi, l):
                """Precompute df for layer l>=1 into SBUF (off-critical)."""
                st = gl[gi]
                NG = st["NG"]
                res = []
                for c in range(2):
                    dfp = ps_pool(gi, c).tile([128, 4 * N], f32, tag="df",
                                              name="dfpL")[:, 0:NG]
                    nc.tensor.matmul(dfp, wdf_l(l, c), st["dsTa"],
                                     start=True, stop=False)
                    nc.tensor.matmul(dfp, wdf_l(l, c), st["dsTb"],
                                     start=False, stop=True)
                    ds = work.tile([128, N], f16, tag=f"gldf{c}{l}",
                                   name="gldfs", bufs=1)[:, 0:NG]
                    nc.scalar.activation(out=ds, in_=dfp, func=AF.Identity,
                                         bias=bdf_l(l, c))
                    res.append(ds)
                st[f"dfs{l}"] = res

            def emit_glast_layer(gi, l):
                st = gl[gi]
                NG = st["NG"]
                ms = []
                for c in range(2):
                    m = work.tile([128, N], f16, tag=f"glm{c}",
                                  name="glm", bufs=2)[:, 0:NG]
                    if l == 0:
                        # df in PSUM; bias + mul fused on DVE; cf from SBUF
                        dfp = ps_pool(gi, c).tile([128, 4 * N], f32, tag="df",
                                                  name="dfpL")[:, 0:NG]
                        nc.tensor.matmul(dfp, wdf_l(l, c), st["dsTa"],
                                         start=True, stop=False)
                        nc.tensor.matmul(dfp, wdf_l(l, c), st["dsTb"],
                                         start=False, stop=True)
                        nc.vector.scalar_tensor_tensor(
                            out=m, in0=dfp, scalar=bdf_l(l, c),
                            in1=st["cfs0"][c],
                            op0=mybir.AluOpType.add, op1=mybir.AluOpType.mult)
                    else:
                        # cf in PSUM; bias + mul fused; df from SBUF
                        nc.vector.scalar_tensor_tensor(
                            out=m, in0=st["cfp"][c], scalar=bcf_l(l, c),
                            in1=st[f"dfs{l}"][c],
                            op0=mybir.AluOpType.add, op1=mybir.AluOpType.mult)
                    ms.append(m)
                hp = (psA if l % 2 == 0 else psB).tile(
                    [F, 4 * N], f32, tag="h", name="hpL")[:, 0:NG]
                nc.tensor.matmul(hp, wfc_l(l, 0), ms[0], start=True, stop=False)
                nc.tensor.matmul(hp, wfc_l(l, 1), ms[1], start=False, stop=True)
                if l < L - 1:
                    hsb = work.tile([F, N], f16, tag="glhsb", name="glhsb",
                                    bufs=2)[:, 0:NG]
                    nc.scalar.activation(out=hsb, in_=hp, func=AF.Copy)
                    th = work.tile([F, N], f16, tag="glth", name="glth",
                                   bufs=2)[:, 0:NG]
                    nc.scalar.activation(out=th, in_=hp, func=AF.Tanh)
                    cfps = []
                    for c in range(2):
                        cfp = ps_pool(gi, c).tile([128, 4 * N], f32, tag="cf",
                                                  name="cfpL")[:, 0:NG]
                        nc.tensor.matmul(cfp, wcf_l(l + 1, c), hsb,
                                         start=True, stop=False)
                        nc.tensor.matmul(cfp, wcf_l(l + 1, c), th,
                                         start=False, stop=True)
                        cfps.append(cfp)
                    st["cfp"] = cfps
                else:
                    _emit_l2_accum(st["bs"], hp)

            def emit_final_head():
                hd = psS.tile([1, BL], f32, tag="hd", name="hd")
                nc.tensor.matmul(hd, head32, g_c, start=True, stop=False)
                nc.tensor.matmul(hd, head32, g_t, start=False, stop=True)
                nc.scalar.activation(out=out_acc, in_=hd, func=AF.Copy)

            # ---- schedule -----------------------------------------------
            fold_half(0, 0)
            fold_half(0, 1, pop=True, then_dma=((4,),))
            fold_half(1, 0)
            fold_half(1, 1, pop=True, then_dma=((5,),))
            emit_glast_pre(2)
            fold_full(2, then_dma=((6,),))
            fold_full(3, then_dma=((7, 0), (7, 1)))
            emit_trs(0)
            emit_layer(0, 0)
            fold_full(4)
            emit_layer(0, 1)
            fold_full(5)
            emit_layer(0, 2)
            fold_full(6)
            emit_trs(1)
            emit_layer(1, 0)
            fold_half(7, 0)
            emit_layer(1, 1)
            fold_half(7, 1, pop=True)
            emit_layer(1, 2)
            emit_glast_trs(2)
            emit_glast_layer(2, 0)
            emit_glast_dfs(2, 1)
            emit_glast_layer(2, 1)
            emit_glast_dfs(2, 2)
            emit_glast_layer(2, 2)
            emit_final_head()

            nc.sync.dma_start(out=out_ext.rearrange("b o -> o b"), in_=out_acc)

    return nc


def _host_pack(x, Wcf_w, Wcf_b, Wdf_w, Wdf_b, Wfc_w, fc0_w, fc0_b, out_w, out_b):
    f = np.float32
    h = np.float16

    def pack_bf(a):  # (rows, 2K) f16 -> (rows, K) fp32 bit-packed
        return np.ascontiguousarray(a.astype(h)).view(f)

    base = np.zeros((128, WCOLS), f)
    base[:, 0:384] = pack_bf(np.asarray(Wcf_w, f).transpose(2, 0, 1).reshape(128, L * H))
    base[:, 384:768] = pack_bf(
        np.asarray(Wfc_w, f).reshape(L, F, 2, 128).transpose(3, 0, 2, 1).reshape(128, L * 2 * F)
    )
    base[:, BCF_OFF : BCF_OFF + 6] = (
        np.asarray(Wcf_b, f).reshape(L, 2, 128).transpose(2, 0, 1).reshape(128, 6)
    )
    base[:, BDF_OFF : BDF_OFF + 6] = (
        (N * np.asarray(Wdf_b, f)).reshape(L, 2, 128).transpose(2, 0, 1).reshape(128, 6)
    )
    w_head = (np.asarray(out_w, np.float64) @ np.asarray(fc0_w, np.float64))[0]  # (F,)
    base[:, HEAD_OFF] = w_head.astype(f)
    # wdf2: rows (j2*64 + r) both halves = Wdf_w[l, h, r]
    wdf2 = np.zeros((128, L * H), f)
    wt = np.asarray(Wdf_w, f).transpose(2, 0, 1).reshape(R, L * H)
    wdf2[0:R] = wt
    wdf2[R:128] = wt
    base[:, WDF_OFF : WDF_OFF + 384] = pack_bf(wdf2)
    base[:, IDOFF : IDOFF + 64] = pack_bf(np.eye(128, dtype=f))
    b_head = float((np.asarray(out_w, np.float64) @ np.asarray(fc0_b, np.float64)
                    + np.asarray(out_b, np.float64)).reshape(()))

    x_t = np.asarray(x, f).transpose(0, 2, 1)  # (B, F, N)
    wpacks = []
    for i in range(NCORES):
        wp = base.copy()
        wp[:, XOFF : XOFF + BL * N // 2] = pack_bf(
            x_t[i * BL : (i + 1) * BL].transpose(1, 0, 2).reshape(128, BL * N)
        )
        wpacks.append(wp)
    return wpacks, b_head


def run(trace=False, **inputs):
    from concourse.bass_utils import run_bass_kernel_spmd

    dist16 = np.ascontiguousarray(
        np.asarray(inputs["distance"]).astype(np.float16).reshape(B, N, N * R)
    )
    wpacks, b_head = _host_pack(
        inputs["x"], inputs["Wcf_w"], inputs["Wcf_b"], inputs["Wdf_w"], inputs["Wdf_b"],
        inputs["Wfc_w"], inputs["fc0_w"], inputs["fc0_b"], inputs["out_w"], inputs["out_b"],
    )

    if "nc" not in _CACHE:
        nc = _build_program()
        nc.finalize()
        _CACHE["nc"] = nc
    nc = _CACHE["nc"]

    in_maps = []
    for i in range(NCORES):
        in_maps.append({
            "dist": np.ascontiguousarray(dist16[i * BL : (i + 1) * BL]),
            "wpack": wpacks[i],
        })
    res = run_bass_kernel_spmd(nc, in_maps, list(range(NCORES)), trace=trace)
    out = np.concatenate([res.results[i]["out"] for i in range(NCORES)], axis=0)
    out = (out.astype(np.float64) + b_head).astype(np.float32)
    return out, res


def kernel(**inputs):
    out, _ = run(trace=False, **inputs)
    return out
